# revision 1
# baseline (speedup 1.0000x reference)
"""Trainium2 Bass kernel for nn_BilinearPolicy (dense_mlp).

Math (reference):
  ob = trunk_obs(obs)      : [B,256] -> 2048 -> 2048 -> 2048 -> 16384 (ReLU between)
  dl = trunk_dlt(deltas)   : same shapes, different weights
  pred[b,a] = sum_f ob[b, a*512+f] * dl[b, f*32+a]            : [B, 32]

Strategy:
  * Data-parallel over batch: 8 cores x 512 rows, zero collectives.
  * Feature-major activations on chip ([feat(part), batch(free)]), so the
    torch-layout weights [din, dout] are used directly as matmul lhsT tiles
    and no transposes are ever needed. Inputs are transposed on host.
  * bf16 matmuls with fp32 PSUM accumulation. Biases applied during the
    mandatory PSUM->SBUF eviction on the Scalar engine (Relu / Identity).
  * dl's last-layer weight columns are permuted on host from (f,a) to (a,f)
    ordering, so the bilinear diagonal becomes: elementwise multiply of the
    two [16384, 512] outputs, then a segmented 512-row partition reduction.
    The 4 z-tiles of an action are accumulated on the DVE; one one-hot mask
    matmul per action accumulates pred^T [32, 512] in a single PSUM tile.
  * Weight streaming is the supply-critical path (~150 GB/s on one HWDGE
    queue with small lines): weights are pre-grouped on host so each DMA
    moves 4 m-tiles (2 MB, 16 KB/partition lines) and DMAs round-robin
    over the sync + scalar HWDGE queues. Small constants go via gpsimd
    SWDGE so they never queue ahead of weights.
"""

from contextlib import ExitStack

import numpy as np
import ml_dtypes

B, OBS, H, F, A = 4096, 256, 2048, 512, 32
DOUT = F * A            # 16384
NCORES = 8
BPC = B // NCORES       # 512 batch rows per core
P = 128

KT = [2, 16, 16, 16]    # k-tiles per layer
MT = [16, 16, 16, 128]  # m-tiles per layer
GR = [8, 4, 4, 4]       # m-tiles grouped per weight DMA

BF16 = ml_dtypes.bfloat16

# Filled with the BassKernelResults of the most recent run (for test harness).
LAST_RESULTS = None


def _tile_weight(w, G):
    """[D1, D2] fp32 -> [D2/(128G), 128(k), G*D1] bf16. Slice
    [:, (g*Kt + j)*128 : +128] of group tile mtg is the lhsT for
    k-tile j of m-tile mtg*G+g; every partition line is contiguous."""
    d1, d2 = w.shape
    kt, mt = d1 // P, d2 // P
    wt = w.reshape(kt, P, mt, P).transpose(2, 1, 0, 3)      # [mt, k, j, m]
    wt = wt.reshape(mt // G, G, P, kt * P).transpose(0, 2, 1, 3)
    wt = wt.reshape(mt // G, P, G * kt * P)
    return np.ascontiguousarray(wt.astype(BF16))


def _tile_bias(b):
    """[D2] fp32 -> [128, D2/128] fp32; column mt holds bias for m-tile mt
    as a per-partition scalar."""
    return np.ascontiguousarray(b.reshape(-1, P).T.astype(np.float32))


def _build_program():
    import concourse.bass as bass
    import concourse.tile as tile
    from concourse import bacc, mybir
    from concourse.bass import ts

    dt = mybir.dt
    AF = mybir.ActivationFunctionType

    nc = bacc.Bacc(
        "TRN2",
        target_bir_lowering=False,
        debug=False,
        enable_asserts=True,
        num_devices=NCORES,
    )

    def din(name, shape, dtype):
        return nc.dram_tensor(name, shape, dtype, kind="ExternalInput").ap()

    x_d = {
        "o": din("xo", [P, 2 * BPC], dt.bfloat16),
        "d": din("xd", [P, 2 * BPC], dt.bfloat16),
    }
    w_d = {}
    b_d = {}
    for t in ("o", "d"):
        for l in range(4):
            w_d[t, l] = din(f"{t}w{l}", [MT[l] // GR[l], P, GR[l] * KT[l] * P],
                            dt.bfloat16)
            b_d[t, l] = din(f"{t}b{l}", [P, MT[l]], dt.float32)
    masks_d = din("masks", [P, A * A], dt.bfloat16)
    pred_d = nc.dram_tensor("pred", [A, BPC], dt.float32, kind="ExternalOutput").ap()

    with tile.TileContext(nc) as tc, ExitStack() as ctx:
        const = ctx.enter_context(tc.tile_pool(name="const", bufs=1))
        wp = ctx.enter_context(tc.tile_pool(name="wp", bufs=4))
        act = ctx.enter_context(tc.tile_pool(name="act", bufs=1))
        ev = ctx.enter_context(tc.tile_pool(name="ev", bufs=10))
        ps = ctx.enter_context(tc.tile_pool(name="ps", bufs=7, space="PSUM"))
        psp = ctx.enter_context(tc.tile_pool(name="psp", bufs=1, space="PSUM"))

        # weight DMAs round-robin over three HWDGE queues so supply isn't
        # capped by a single queue's descriptor rate
        dma_engs = [nc.sync, nc.scalar]
        rr = [0]

        def wdma(dst, src):
            dma_engs[rr[0] % len(dma_engs)].dma_start(dst, src)
            rr[0] += 1

        # inputs + L0 weights first, in two chunks split across both HWDGE
        # queues in first-use order so the PE can start within a few us;
        # other small constants go on the gpsimd SWDGE queue
        x_sb = {}
        w0_sb = {}
        chunk = GR[0] * KT[0] * P
        for t in ("o", "d"):
            x_sb[t] = const.tile([P, 2, BPC], dt.bfloat16,
                                 tag=f"x{t}", name=f"x{t}")
            wdma(x_sb[t][:], x_d[t].rearrange("p (k n) -> p k n", n=BPC))
            w0_sb[t] = const.tile([P, (MT[0] // GR[0]) * chunk], dt.bfloat16,
                                  tag=f"w0{t}", name=f"w0{t}")
            for c in range(MT[0] // GR[0]):
                wdma(w0_sb[t][:, c * chunk:(c + 1) * chunk], w_d[t, 0][c])
        bias_sb = {}
        for t in ("o", "d"):
            for l in range(4):
                bias_sb[t, l] = const.tile([P, MT[l]], dt.float32,
                                           tag=f"b{t}{l}", name=f"b{t}{l}")
                nc.gpsimd.dma_start(bias_sb[t, l][:], b_d[t, l][:])
        masks_sb = const.tile([P, A * A], dt.bfloat16, tag="masks")
        nc.gpsimd.dma_start(masks_sb[:], masks_d[:])

        # ---- Trunks: layers 0..2 with ReLU, feature-major throughout.
        # The two trunks are interleaved layer-by-layer so the PE has twice
        # the work per phase start, covering the weight-stream warm-up.
        cur = dict(x_sb)
        for l in range(3):
            for t in ("o", "d"):
                out_t = act.tile([P, MT[l], BPC], dt.bfloat16,
                                 tag=f"h{t}{l % 2}", name=f"h{t}{l}")
                for mtg in range(MT[l] // GR[l]):
                    if l == 0:
                        wt = w0_sb[t]
                    else:
                        wt = wp.tile([P, GR[l] * KT[l] * P], dt.bfloat16,
                                     tag="wbig")
                        wdma(wt[:], w_d[t, l][mtg])
                    for g in range(GR[l]):
                        mt = mtg * GR[l] + g
                        wcol = mt if l == 0 else g  # L0 tile is fully resident
                        pt = ps.tile([P, BPC], dt.float32, tag="mm")
                        for j in range(KT[l]):
                            nc.tensor.matmul(
                                pt[:], wt[:, ts(wcol * KT[l] + j, P)],
                                cur[t][:, j, :],
                                start=(j == 0), stop=(j == KT[l] - 1),
                            )
                        nc.scalar.activation(
                            out_t[:, mt, :], pt[:], AF.Relu,
                            bias=bias_sb[t, l][:, mt:mt + 1],
                        )
                cur[t] = out_t
        h = cur

        # ---- Layer 3 + bilinear diagonal, fused per 128-feature tile.
        pred_ps = psp.tile([A, BPC], dt.float32, tag="pred")
        for a in range(A):  # one weight DMA per trunk covers the whole action
            z_acc = ev.tile([P, BPC], dt.bfloat16, tag="zacc")
            wt = {}
            for t in ("o", "d"):
                wt[t] = wp.tile([P, GR[3] * KT[3] * P], dt.bfloat16,
                                tag="wbig", name=f"w3{t}")
                wdma(wt[t][:], w_d[t, 3][a])
            for g in range(GR[3]):
                mt = a * 4 + g
                s = {}
                for t in ("o", "d"):
                    pt = ps.tile([P, BPC], dt.float32, tag="mm")
                    for j in range(KT[3]):
                        nc.tensor.matmul(
                            pt[:], wt[t][:, ts(g * KT[3] + j, P)],
                            h[t][:, j, :],
                            start=(j == 0), stop=(j == KT[3] - 1),
                        )
                    s[t] = ev.tile([P, BPC], dt.bfloat16, tag="evict",
                                   name=f"s{t}")
                    nc.scalar.activation(
                        s[t][:], pt[:], AF.Identity,
                        bias=bias_sb[t, 3][:, mt:mt + 1],
                    )
                if g == 0:
                    nc.vector.tensor_mul(z_acc[:], s["o"][:], s["d"][:])
                else:
                    zt = ev.tile([P, BPC], dt.bfloat16, tag="ztmp")
                    nc.vector.tensor_mul(zt[:], s["o"][:], s["d"][:])
                    nc.vector.tensor_add(z_acc[:], z_acc[:], zt[:])
            nc.tensor.matmul(
                pred_ps[:], masks_sb[:, ts(a, A)], z_acc[:],
                start=(a == 0), stop=(a == A - 1),
            )

        pred_sb = ev.tile([A, BPC], dt.float32, tag="predsb")
        nc.vector.tensor_copy(pred_sb[:], pred_ps[:])
        nc.sync.dma_start(pred_d[:], pred_sb[:])

    nc.compile()
    return nc


def _prep_inputs(inputs):
    """Host-side layout/dtype prep shared across cores + per-core slices."""
    shared = {}

    for t, pfx in (("o", "obs"), ("d", "dlt")):
        for l in range(4):
            w = np.asarray(inputs[f"{pfx}_W{l}"], np.float32)
            b = np.asarray(inputs[f"{pfx}_b{l}"], np.float32)
            if t == "d" and l == 3:
                # permute columns (f,a) -> (a,f) to match obs layout
                w = w.reshape(H, F, A).transpose(0, 2, 1).reshape(H, DOUT)
                b = b.reshape(F, A).T.reshape(DOUT)
            shared[f"{t}w{l}"] = _tile_weight(w, GR[l])
            shared[f"{t}b{l}"] = _tile_bias(b)

    masks = np.zeros((P, A, A), np.float32)
    for a in range(A):
        masks[:, a, a] = 1.0
    shared["masks"] = np.ascontiguousarray(masks.reshape(P, A * A).astype(BF16))

    obsT = np.asarray(inputs["obs"], np.float32).T.astype(BF16)    # [256, 4096]
    dltT = np.asarray(inputs["deltas"], np.float32).T.astype(BF16)

    in_maps = []
    for c in range(NCORES):
        sl = slice(c * BPC, (c + 1) * BPC)
        m = dict(shared)
        m["xo"] = np.ascontiguousarray(
            obsT[:, sl].reshape(2, P, BPC).transpose(1, 0, 2).reshape(P, 2 * BPC))
        m["xd"] = np.ascontiguousarray(
            dltT[:, sl].reshape(2, P, BPC).transpose(1, 0, 2).reshape(P, 2 * BPC))
        in_maps.append(m)
    return in_maps


_PROGRAM = None


def kernel(**inputs):
    global _PROGRAM, LAST_RESULTS
    from concourse.bass_utils import run_bass_kernel_spmd

    if _PROGRAM is None:
        _PROGRAM = _build_program()
    in_maps = _prep_inputs(inputs)
    res = run_bass_kernel_spmd(_PROGRAM, in_maps, list(range(NCORES)))
    LAST_RESULTS = res
    out = np.empty((B, A), np.float32)
    for c in range(NCORES):
        out[c * BPC:(c + 1) * BPC] = res.results[c]["pred"].T
    return out



# revision 6
# speedup vs baseline: 1.0856x; 1.0856x over previous
"""Trainium2 Bass kernel for nn_BilinearPolicy (dense_mlp).

Math (reference):
  ob = trunk_obs(obs)      : [B,256] -> 2048 -> 2048 -> 2048 -> 16384 (ReLU between)
  dl = trunk_dlt(deltas)   : same shapes, different weights
  pred[b,a] = sum_f ob[b, a*512+f] * dl[b, f*32+a]            : [B, 32]

Strategy:
  * Data-parallel over batch: 8 cores x 512 rows, zero collectives.
  * Feature-major activations on chip ([feat(part), batch(free)]), so the
    torch-layout weights [din, dout] are used directly as matmul lhsT tiles
    and no transposes are ever needed. Inputs are transposed on host.
  * bf16 matmuls with fp32 PSUM accumulation. Biases applied during the
    mandatory PSUM->SBUF eviction on the Scalar engine (Relu / Identity).
  * Mixed-precision L3 (79% of the FLOPs): L2's output features are
    permuted on host (free: permute W2 columns + W3 rows) so the Q coldest
    128-feature blocks (host-calibrated power on a 256-row subsample) come
    first; those blocks are quantized to fp8-e4m3 and contracted with
    DoubleRow matmuls (2 k-blocks per MM at ~0.5 cycles/row). The L3 PSUM
    runs in G*y units (bf16 W3 tiles scaled by G on host; fp8 products
    carry G via the weight scale G/s against the activation scale s);
    evictions apply 1/G before the bias.
  * dl's last-layer weight columns are permuted on host from (f,a) to (a,f)
    ordering, so the bilinear diagonal becomes an elementwise multiply of
    the two trunk outputs + segmented reduction. The z path (evict, mul,
    accumulate) runs in fp32; one one-hot mask matmul per action (fp32r,
    full rate at N=512) accumulates pred^T [32, 512] in a single PSUM tile.
  * Weight streaming: weights pre-grouped on host so each DMA moves large
    contiguous partition lines, round-robin over the sync + scalar HWDGE
    queues. Small constants go via gpsimd SWDGE.
"""

from contextlib import ExitStack

import numpy as np
import ml_dtypes

B, OBS, H, F, A = 4096, 256, 2048, 512, 32
DOUT = F * A            # 16384
NCORES = 8
BPC = B // NCORES       # 512 batch rows per core
P = 128

KT = [2, 16, 16, 16]    # k-tiles per layer
MT = [16, 16, 16, 128]  # m-tiles per layer
GR = [8, 4, 4, 4]       # m-tiles grouped per weight DMA

Q = 8                   # L3 k-blocks (of 16) quantized to fp8-DoubleRow
NB = 16 - Q             # L3 k-blocks kept in bf16
G = 4096.0              # L3 PSUM scale
CAL_ROWS = 256          # host calibration subsample
HMARGIN = 120.0         # fp8 |h*s| target max (overflow at 240)

F16 = np.float16
E4M3 = ml_dtypes.float8_e4m3   # IEEE-style: max 240, matches TRN FP8_EXP4

# Filled with the BassKernelResults of the most recent run (for test harness).
LAST_RESULTS = None


def _tile_weight(w, G_):
    """[D1, D2] fp32 -> [D2/(128G), 128(k), G*D1] bf16. Slice
    [:, (g*Kt + j)*128 : +128] of group tile mtg is the lhsT for
    k-tile j of m-tile mtg*G+g; every partition line is contiguous."""
    d1, d2 = w.shape
    kt, mt = d1 // P, d2 // P
    wt = w.reshape(kt, P, mt, P).transpose(2, 1, 0, 3)      # [mt, k, j, m]
    wt = wt.reshape(mt // G_, G_, P, kt * P).transpose(0, 2, 1, 3)
    wt = wt.reshape(mt // G_, P, G_ * kt * P)
    return np.ascontiguousarray(wt.astype(F16))


def _tile_bias(b):
    """[D2] fp32 -> [128, D2/128] fp32; column mt holds bias for m-tile mt
    as a per-partition scalar."""
    return np.ascontiguousarray(b.reshape(-1, P).T.astype(np.float32))


def _build_program():
    import concourse.bass as bass
    import concourse.tile as tile
    from concourse import bacc, mybir
    from concourse.bass import ts

    dt = mybir.dt
    AF = mybir.ActivationFunctionType
    DRM = mybir.MatmulPerfMode.DoubleRow

    nc = bacc.Bacc(
        "TRN2",
        target_bir_lowering=False,
        debug=False,
        enable_asserts=True,
        num_devices=NCORES,
    )

    def din(name, shape, dtype):
        return nc.dram_tensor(name, shape, dtype, kind="ExternalInput").ap()

    x_d = {
        "o": din("xo", [P, 2 * BPC], dt.float16),
        "d": din("xd", [P, 2 * BPC], dt.float16),
    }
    w_d = {}
    b_d = {}
    for t in ("o", "d"):
        for l in range(3):
            w_d[t, l] = din(f"{t}w{l}", [MT[l] // GR[l], P, GR[l] * KT[l] * P],
                            dt.float16)
        # L3 split: bf16 part (NB k-blocks) + fp8 DoubleRow part (Q k-blocks)
        w_d[t, "3b"] = din(f"{t}w3b", [A, P, GR[3] * NB * P], dt.float16)
        w_d[t, "3q"] = din(f"{t}w3q", [A, P, Q * GR[3] * P], dt.float8e4)
        for l in range(4):
            b_d[t, l] = din(f"{t}b{l}", [P, MT[l]], dt.float32)
    masks_d = din("masks", [P, A * A], dt.float16)
    scales_d = din("scales", [P, 2], dt.float32)
    pred_d = nc.dram_tensor("pred", [A, BPC], dt.float32, kind="ExternalOutput").ap()

    with tile.TileContext(nc) as tc, ExitStack() as ctx:
        const = ctx.enter_context(tc.tile_pool(name="const", bufs=1))
        wp = ctx.enter_context(tc.tile_pool(name="wp", bufs=3))
        wp3 = ctx.enter_context(tc.tile_pool(name="wp3", bufs=3))
        act = ctx.enter_context(tc.tile_pool(name="act", bufs=1))
        ev = ctx.enter_context(tc.tile_pool(name="ev", bufs=3))
        ps = ctx.enter_context(tc.tile_pool(name="ps", bufs=7, space="PSUM"))
        psp = ctx.enter_context(tc.tile_pool(name="psp", bufs=1, space="PSUM"))

        # weight DMAs round-robin over the sync + scalar HWDGE queues so
        # supply isn't capped by a single queue's descriptor rate
        dma_engs = [nc.sync, nc.scalar]
        rr = [0]

        def wdma(dst, src):
            dma_engs[rr[0] % len(dma_engs)].dma_start(dst, src)
            rr[0] += 1

        # inputs + L0 weights first, split across both HWDGE queues in
        # first-use order so the PE can start within a few us; other small
        # constants go on the gpsimd SWDGE queue
        x_sb = {}
        w0_sb = {}
        chunk = GR[0] * KT[0] * P
        for t in ("o", "d"):
            x_sb[t] = const.tile([P, 2, BPC], dt.float16,
                                 tag=f"x{t}", name=f"x{t}")
            wdma(x_sb[t][:], x_d[t].rearrange("p (k n) -> p k n", n=BPC))
            w0_sb[t] = const.tile([P, (MT[0] // GR[0]) * chunk], dt.float16,
                                  tag=f"w0{t}", name=f"w0{t}")
            for c in range(MT[0] // GR[0]):
                wdma(w0_sb[t][:, c * chunk:(c + 1) * chunk], w_d[t, 0][c])
        bias_sb = {}
        for t in ("o", "d"):
            for l in range(4):
                bias_sb[t, l] = const.tile([P, MT[l]], dt.float32,
                                           tag=f"b{t}{l}", name=f"b{t}{l}")
                nc.gpsimd.dma_start(bias_sb[t, l][:], b_d[t, l][:])
        masks_sb = const.tile([P, A * A], dt.float16, tag="masks")
        nc.gpsimd.dma_start(masks_sb[:], masks_d[:])
        scales_sb = const.tile([P, 2], dt.float32, tag="scales")
        nc.gpsimd.dma_start(scales_sb[:], scales_d[:])
        scol = {"o": 0, "d": 1}

        # ---- Trunks: layers 0..2 with ReLU, feature-major throughout.
        # The two trunks are interleaved layer-by-layer so the PE has twice
        # the work per phase start, covering the weight-stream warm-up.
        # L2's output features are host-permuted: blocks 0..Q-1 (coldest)
        # are evicted to fp8 with scale s_t, blocks Q..15 to bf16.
        cur = dict(x_sb)
        hq_sb = {}
        for l in range(3):
            for t in ("o", "d"):
                out_t = act.tile([P, MT[l], BPC], dt.float16,
                                 tag=f"h{t}{l % 2}", name=f"h{t}{l}")
                if l == 2:
                    hq_sb[t] = act.tile([P, Q, BPC], dt.float8e4,
                                        tag=f"hq{t}", name=f"hq{t}")
                for mtg in range(MT[l] // GR[l]):
                    if l == 0:
                        wt = w0_sb[t]
                    else:
                        wt = wp.tile([P, GR[l] * KT[l] * P], dt.float16,
                                     tag="wbig")
                        wdma(wt[:], w_d[t, l][mtg])
                    for g in range(GR[l]):
                        mt = mtg * GR[l] + g
                        wcol = mt if l == 0 else g  # L0 tile is fully resident
                        pt = ps.tile([P, BPC], dt.float32, tag="mm")
                        for j in range(KT[l]):
                            nc.tensor.matmul(
                                pt[:], wt[:, ts(wcol * KT[l] + j, P)],
                                cur[t][:, j, :],
                                start=(j == 0), stop=(j == KT[l] - 1),
                            )
                        if l == 2 and mt < Q:
                            # Relu(psum*s + s*b) = s*Relu(psum+b) -> fp8
                            nc.scalar.activation(
                                hq_sb[t][:, mt, :], pt[:], AF.Relu,
                                bias=bias_sb[t, 2][:, mt:mt + 1],
                                scale=scales_sb[:, scol[t]:scol[t] + 1],
                            )
                        else:
                            slot = mt if l < 2 else mt - Q
                            nc.scalar.activation(
                                out_t[:, slot, :], pt[:], AF.Relu,
                                bias=bias_sb[t, l][:, mt:mt + 1],
                            )
                cur[t] = out_t
        h = cur

        # interleaved L3 MM sequence: every DoubleRow LDWEIGHTS (no FWL,
        # 256 cols) hides under a long predecessor stream
        seq = []
        for j in range(max(NB, Q // 2)):
            if j < NB:
                seq.append(("b", j))
            if j < Q // 2:
                seq.append(("q", j))

        # ---- Layer 3 + bilinear diagonal, fused per 128-feature tile.
        pred_ps = psp.tile([A, BPC], dt.float32, tag="pred")
        for a in range(A):  # one weight DMA pair per trunk covers the action
            wtb = {}
            wtq = {}
            for t in ("o", "d"):
                wtb[t] = wp3.tile([P, GR[3] * NB * P], dt.float16,
                                  tag="w3b", name=f"w3b{t}")
                wdma(wtb[t][:], w_d[t, "3b"][a])
                wtq[t] = wp3.tile([P, Q, GR[3] * P], dt.float8e4,
                                  tag="w3q", name=f"w3q{t}")
                wdma(wtq[t][:], w_d[t, "3q"][a].rearrange(
                    "p (q m) -> p q m", q=Q))
            zt = {}
            for g in range(GR[3]):
                mt = a * 4 + g
                s = {}
                for t in ("o", "d"):
                    pt = ps.tile([P, BPC], dt.float32, tag="mm")
                    for i, (kind, j) in enumerate(seq):
                        if kind == "b":
                            nc.tensor.matmul(
                                pt[:], wtb[t][:, ts(g * NB + j, P)],
                                h[t][:, j, :],
                                start=(i == 0), stop=(i == len(seq) - 1),
                            )
                        else:
                            nc.tensor.matmul(
                                pt[:], wtq[t][:, 2 * j:2 * j + 2, ts(g, P)],
                                hq_sb[t][:, 2 * j:2 * j + 2, :],
                                start=(i == 0), stop=(i == len(seq) - 1),
                                perf_mode=DRM,
                            )
                    s[t] = ev.tile([P, BPC], dt.float16, tag="evict",
                                   name=f"s{t}")
                    nc.scalar.activation(
                        s[t][:], pt[:], AF.Identity,
                        bias=bias_sb[t, 3][:, mt:mt + 1],
                        scale=1.0 / G,
                    )
                zt[g] = ev.tile([P, BPC], dt.float16, tag=f"z{g % 2}",
                                name=f"z{g}")
                nc.vector.tensor_mul(zt[g][:], s["o"][:], s["d"][:])
                if g == 1:
                    nc.vector.tensor_add(zt[0][:], zt[0][:], zt[1][:])
                if g == 3:
                    nc.vector.tensor_add(zt[2][:], zt[2][:], zt[3][:])
                    nc.vector.tensor_add(zt[0][:], zt[0][:], zt[2][:])
            nc.tensor.matmul(
                pred_ps[:], masks_sb[:, ts(a, A)], zt[0][:],
                start=(a == 0), stop=(a == A - 1),
            )

        pred_sb = ev.tile([A, BPC], dt.float32, tag="predsb")
        nc.vector.tensor_copy(pred_sb[:], pred_ps[:])
        nc.sync.dma_start(pred_d[:], pred_sb[:])

    nc.compile()
    return nc


def _calibrate(inputs):
    """Host calibration: per-trunk L2-output feature power + max on a
    CAL_ROWS-row subsample. Returns per-trunk (perm, s)."""
    out = {}
    for t, pfx, xk in (("o", "obs", "obs"), ("d", "dlt", "deltas")):
        hc = np.asarray(inputs[xk][:CAL_ROWS], np.float32)
        for l in range(3):
            W = np.asarray(inputs[f"{pfx}_W{l}"], np.float32)
            b = np.asarray(inputs[f"{pfx}_b{l}"], np.float32)
            hc = np.maximum(hc @ W + b, 0.0)
        power = (hc ** 2).mean(0)
        perm = np.argsort(power)
        hmax = hc[:, perm[:Q * P]].max()
        out[t] = (perm, HMARGIN / max(hmax, 1e-9))
    return out


def _prep_inputs(inputs):
    """Host-side layout/dtype prep shared across cores + per-core slices."""
    shared = {}
    cal = _calibrate(inputs)

    for t, pfx in (("o", "obs"), ("d", "dlt")):
        perm, s = cal[t]
        for l in range(3):
            w = np.asarray(inputs[f"{pfx}_W{l}"], np.float32)
            b = np.asarray(inputs[f"{pfx}_b{l}"], np.float32)
            if l == 2:
                w = w[:, perm]
                b = b[perm]
            shared[f"{t}w{l}"] = _tile_weight(w, GR[l])
            bt = _tile_bias(b)
            if l == 2:
                bt[:, :Q] *= s  # fp8 evictions get scale-folded biases
            shared[f"{t}b{l}"] = bt

        w3 = np.asarray(inputs[f"{pfx}_W3"], np.float32)
        b3 = np.asarray(inputs[f"{pfx}_b3"], np.float32)
        if t == "d":
            # permute columns (f,a) -> (a,f) to match obs layout
            w3 = w3.reshape(H, F, A).transpose(0, 2, 1).reshape(H, DOUT)
            b3 = b3.reshape(F, A).T.reshape(DOUT)
        w3 = w3[perm, :]
        # bf16 part: k-blocks Q..15, scaled by G; column order within an
        # action must be (g*NB + j)*128 + m
        w3b = w3[Q * P:, :] * G                         # [NB*128, 16384]
        w3b = w3b.reshape(NB, P, A, GR[3], P)           # [j, k, a, g, m]
        w3b = w3b.transpose(2, 1, 3, 0, 4)              # [a, k, g, j, m]
        w3b = w3b.reshape(A, P, GR[3] * NB * P)
        shared[f"{t}w3b"] = np.ascontiguousarray(w3b.astype(F16))
        # fp8 part: k-blocks 0..Q-1, scaled by G/s; layout [A, P, Q, 4*128]
        w3q = w3[:Q * P, :] * (G / s)                   # [Q*128, 16384]
        assert np.abs(w3q).max() < 240.0, np.abs(w3q).max()
        w3q = w3q.reshape(Q, P, A, GR[3] * P)           # [j, k, a, gm]
        w3q = w3q.transpose(2, 1, 0, 3)                 # [a, k, j, gm]
        w3q = w3q.reshape(A, P, Q * GR[3] * P)
        shared[f"{t}w3q"] = np.ascontiguousarray(w3q.astype(E4M3))
        shared[f"{t}b3"] = _tile_bias(b3)

    masks = np.zeros((P, A, A), np.float32)
    for a in range(A):
        masks[:, a, a] = 1.0
    shared["masks"] = np.ascontiguousarray(masks.reshape(P, A * A).astype(F16))
    shared["scales"] = np.ascontiguousarray(
        np.broadcast_to(np.array([cal["o"][1], cal["d"][1]], np.float32),
                        (P, 2)))

    obsT = np.asarray(inputs["obs"], np.float32).T.astype(F16)    # [256, 4096]
    dltT = np.asarray(inputs["deltas"], np.float32).T.astype(F16)

    in_maps = []
    for c in range(NCORES):
        sl = slice(c * BPC, (c + 1) * BPC)
        m = dict(shared)
        m["xo"] = np.ascontiguousarray(
            obsT[:, sl].reshape(2, P, BPC).transpose(1, 0, 2).reshape(P, 2 * BPC))
        m["xd"] = np.ascontiguousarray(
            dltT[:, sl].reshape(2, P, BPC).transpose(1, 0, 2).reshape(P, 2 * BPC))
        in_maps.append(m)
    return in_maps


_PROGRAM = None


def kernel(**inputs):
    global _PROGRAM, LAST_RESULTS
    from concourse.bass_utils import run_bass_kernel_spmd

    if _PROGRAM is None:
        _PROGRAM = _build_program()
    in_maps = _prep_inputs(inputs)
    res = run_bass_kernel_spmd(_PROGRAM, in_maps, list(range(NCORES)))
    LAST_RESULTS = res
    out = np.empty((B, A), np.float32)
    for c in range(NCORES):
        out[c * BPC:(c + 1) * BPC] = res.results[c]["pred"].T
    return out


# revision 9
# speedup vs baseline: 1.2012x; 1.1065x over previous
"""Trainium2 Bass kernel for nn_BilinearPolicy (dense_mlp).

Math (reference):
  ob = trunk_obs(obs)      : [B,256] -> 2048 -> 2048 -> 2048 -> 16384 (ReLU between)
  dl = trunk_dlt(deltas)   : same shapes, different weights
  pred[b,a] = sum_f ob[b, a*512+f] * dl[b, f*32+a]            : [B, 32]

Strategy:
  * Data-parallel over batch: 8 cores x 512 rows, zero collectives.
  * Feature-major activations on chip ([feat(part), batch(free)]), so the
    torch-layout weights [din, dout] are used directly as matmul lhsT tiles
    and no transposes are ever needed. Inputs are transposed on host.
  * fp16 matmuls with fp32 PSUM accumulation (same PE rate as bf16,
    8x finer mantissa). Biases applied during the
    mandatory PSUM->SBUF eviction on the Scalar engine (Relu / Identity).
  * Mixed-precision L3 (79% of the FLOPs): L2's output features are
    permuted on host (free: permute W2 columns + W3 rows) so the Q coldest
    128-feature blocks (host-calibrated power on a 256-row subsample) come
    first; those blocks are quantized to fp8-e4m3 and contracted with
    DoubleRow matmuls (2 k-blocks per MM at ~0.5 cycles/row). The L3 PSUM
    runs in G*y units (fp16 W3 tiles scaled by G on host; fp8 products
    carry G via the weight scale G/s against the activation scale s);
    evictions apply 1/G before the bias.
  * dl's last-layer weight columns are permuted on host from (f,a) to (a,f)
    ordering, so the bilinear diagonal becomes an elementwise multiply of
    the two trunk outputs + segmented reduction. The z path (evict, mul,
    accumulate) runs in fp16 (2x DVE rate); one one-hot mask matmul per
    action accumulates pred^T [32, 512] in a single PSUM tile.
  * Weight streaming: weights pre-grouped on host so each DMA moves large
    contiguous partition lines, round-robin over the sync + scalar HWDGE
    queues. Small constants go via gpsimd SWDGE.
"""

from contextlib import ExitStack

import numpy as np
import ml_dtypes

B, OBS, H, F, A = 4096, 256, 2048, 512, 32
DOUT = F * A            # 16384
NCORES = 8
BPC = B // NCORES       # 512 batch rows per core
P = 128

KT = [2, 16, 16, 16]    # k-tiles per layer
MT = [16, 16, 16, 128]  # m-tiles per layer
GR = [8, 4, 4, 4]       # m-tiles grouped per weight DMA

Q = 8                   # L3 k-blocks (of 16) quantized to fp8-DoubleRow
NB = 16 - Q             # L3 k-blocks kept in fp16
G = 4096.0              # L3 PSUM scale
CAL_ROWS = 256          # host calibration subsample
HMARGIN = 120.0         # fp8 |h*s| target max (overflow at 240)

F16 = np.float16
E4M3 = ml_dtypes.float8_e4m3   # IEEE-style: max 240, matches TRN FP8_EXP4

# Filled with the BassKernelResults of the most recent run (for test harness).
LAST_RESULTS = None


def _tile_weight(w, G_):
    """[D1, D2] fp32 -> [D2/(128G), 128(k), G*D1] bf16. Slice
    [:, (g*Kt + j)*128 : +128] of group tile mtg is the lhsT for
    k-tile j of m-tile mtg*G+g; every partition line is contiguous."""
    d1, d2 = w.shape
    kt, mt = d1 // P, d2 // P
    wt = w.reshape(kt, P, mt, P).transpose(2, 1, 0, 3)      # [mt, k, j, m]
    wt = wt.reshape(mt // G_, G_, P, kt * P).transpose(0, 2, 1, 3)
    wt = wt.reshape(mt // G_, P, G_ * kt * P)
    return np.ascontiguousarray(wt.astype(F16))


def _tile_bias(b):
    """[D2] fp32 -> [128, D2/128] fp32; column mt holds bias for m-tile mt
    as a per-partition scalar."""
    return np.ascontiguousarray(b.reshape(-1, P).T.astype(np.float32))


def _build_program():
    import concourse.bass as bass
    import concourse.tile as tile
    from concourse import bacc, mybir
    from concourse.bass import ts

    dt = mybir.dt
    AF = mybir.ActivationFunctionType
    DRM = mybir.MatmulPerfMode.DoubleRow

    nc = bacc.Bacc(
        "TRN2",
        target_bir_lowering=False,
        debug=False,
        enable_asserts=True,
        num_devices=NCORES,
    )

    def din(name, shape, dtype):
        return nc.dram_tensor(name, shape, dtype, kind="ExternalInput").ap()

    x_d = {
        "o": din("xo", [P, 2 * BPC], dt.float16),
        "d": din("xd", [P, 2 * BPC], dt.float16),
    }
    w_d = {}
    b_d = {}
    for t in ("o", "d"):
        for l in range(3):
            w_d[t, l] = din(f"{t}w{l}", [MT[l] // GR[l], P, GR[l] * KT[l] * P],
                            dt.float16)
        # L3 split: fp16 part (NB k-blocks) + fp8 DoubleRow part (Q k-blocks)
        w_d[t, "3b"] = din(f"{t}w3b", [A, P, GR[3] * NB * P], dt.float16)
        w_d[t, "3q"] = din(f"{t}w3q", [A, P, Q * GR[3] * P], dt.float8e4)
        for l in range(4):
            b_d[t, l] = din(f"{t}b{l}", [P, MT[l]], dt.float32)
    masks_d = din("masks", [P, A * A], dt.float16)
    scales_d = din("scales", [P, 2], dt.float32)
    pred_d = nc.dram_tensor("pred", [A, BPC], dt.float32, kind="ExternalOutput").ap()

    with tile.TileContext(nc) as tc, ExitStack() as ctx:
        const = ctx.enter_context(tc.tile_pool(name="const", bufs=1))
        wp = ctx.enter_context(tc.tile_pool(name="wp", bufs=4))
        wp3 = ctx.enter_context(tc.tile_pool(name="wp3", bufs=4))
        act = ctx.enter_context(tc.tile_pool(name="act", bufs=1))
        ev = ctx.enter_context(tc.tile_pool(name="ev", bufs=3))
        ps = ctx.enter_context(tc.tile_pool(name="ps", bufs=7, space="PSUM"))
        psp = ctx.enter_context(tc.tile_pool(name="psp", bufs=1, space="PSUM"))

        # weight DMAs round-robin over the sync + scalar HWDGE queues so
        # supply isn't capped by a single queue's descriptor rate
        dma_engs = [nc.sync, nc.scalar]
        rr = [0]

        def wdma(dst, src):
            dma_engs[rr[0] % len(dma_engs)].dma_start(dst, src)
            rr[0] += 1

        # inputs + L0 weights first, split across both HWDGE queues in
        # first-use order so the PE can start within a few us; other small
        # constants go on the gpsimd SWDGE queue
        x_sb = {}
        for t in ("o", "d"):
            x_sb[t] = const.tile([P, 2, BPC], dt.float16,
                                 tag=f"x{t}", name=f"x{t}")
            xsrc = x_d[t].rearrange("p (k n) -> p k n", n=BPC)
            for k in range(2):
                wdma(x_sb[t][:, k, :], xsrc[:, k, :])
        bias_sb = {}
        for t in ("o", "d"):
            for l in range(4):
                bias_sb[t, l] = const.tile([P, MT[l]], dt.float32,
                                           tag=f"b{t}{l}", name=f"b{t}{l}")
                nc.gpsimd.dma_start(bias_sb[t, l][:], b_d[t, l][:])
        masks_sb = const.tile([P, A * A], dt.float16, tag="masks")
        nc.gpsimd.dma_start(masks_sb[:], masks_d[:])
        scales_sb = const.tile([P, 2], dt.float32, tag="scales")
        nc.gpsimd.dma_start(scales_sb[:], scales_d[:])
        scol = {"o": 0, "d": 1}

        # ---- Trunks: layers 0..2 with ReLU, feature-major throughout.
        # The two trunks are interleaved layer-by-layer so the PE has twice
        # the work per phase start, covering the weight-stream warm-up.
        # L2's output features are host-permuted: blocks 0..Q-1 (coldest)
        # are evicted to fp8 with scale s_t, blocks Q..15 to fp16.
        cur = dict(x_sb)
        hq_sb = {}
        for l in range(3):
            for t in ("o", "d"):
                out_t = act.tile([P, MT[l], BPC], dt.float16,
                                 tag=f"h{t}{l % 2}", name=f"h{t}{l}")
                if l == 2:
                    hq_sb[t] = act.tile([P, Q, BPC], dt.float8e4,
                                        tag=f"hq{t}", name=f"hq{t}")
                for mtg in range(MT[l] // GR[l]):
                    wt = wp.tile([P, GR[l] * KT[l] * P], dt.float16,
                                 tag="wbig")
                    wdma(wt[:], w_d[t, l][mtg])
                    for g in range(GR[l]):
                        mt = mtg * GR[l] + g
                        wcol = g
                        pt = ps.tile([P, BPC], dt.float32, tag="mm")
                        for j in range(KT[l]):
                            nc.tensor.matmul(
                                pt[:], wt[:, ts(wcol * KT[l] + j, P)],
                                cur[t][:, j, :],
                                start=(j == 0), stop=(j == KT[l] - 1),
                            )
                        if l == 2 and mt < Q:
                            # Relu(psum*s + s*b) = s*Relu(psum+b) -> fp8
                            nc.scalar.activation(
                                hq_sb[t][:, mt, :], pt[:], AF.Relu,
                                bias=bias_sb[t, 2][:, mt:mt + 1],
                                scale=scales_sb[:, scol[t]:scol[t] + 1],
                            )
                        else:
                            slot = mt if l < 2 else mt - Q
                            nc.scalar.activation(
                                out_t[:, slot, :], pt[:], AF.Relu,
                                bias=bias_sb[t, l][:, mt:mt + 1],
                            )
                cur[t] = out_t
        h = cur

        # interleaved L3 MM sequence: every DoubleRow LDWEIGHTS (no FWL,
        # 256 cols) hides under a long predecessor stream
        seq = []
        for j in range(max(NB, Q // 2)):
            if j < NB:
                seq.append(("b", j))
            if j < Q // 2:
                seq.append(("q", j))

        # ---- Layer 3 + bilinear diagonal, fused per 128-feature tile.
        pred_ps = psp.tile([A, BPC], dt.float32, tag="pred")
        for a in range(A):  # one weight DMA pair per trunk covers the action
            wtb = {}
            wtq = {}
            for t in ("o", "d"):
                wtb[t] = wp3.tile([P, GR[3] * NB * P], dt.float16,
                                  tag="w3b", name=f"w3b{t}")
                wdma(wtb[t][:], w_d[t, "3b"][a])
                wtq[t] = wp3.tile([P, Q, GR[3] * P], dt.float8e4,
                                  tag="w3q", name=f"w3q{t}")
                nc.gpsimd.dma_start(wtq[t][:], w_d[t, "3q"][a].rearrange(
                    "p (q m) -> p q m", q=Q))
            zt = {}
            for g in range(GR[3]):
                mt = a * 4 + g
                s = {}
                for t in ("o", "d"):
                    pt = ps.tile([P, BPC], dt.float32, tag="mm")
                    for i, (kind, j) in enumerate(seq):
                        if kind == "b":
                            nc.tensor.matmul(
                                pt[:], wtb[t][:, ts(g * NB + j, P)],
                                h[t][:, j, :],
                                start=(i == 0), stop=(i == len(seq) - 1),
                            )
                        else:
                            nc.tensor.matmul(
                                pt[:], wtq[t][:, 2 * j:2 * j + 2, ts(g, P)],
                                hq_sb[t][:, 2 * j:2 * j + 2, :],
                                start=(i == 0), stop=(i == len(seq) - 1),
                                perf_mode=DRM,
                            )
                    s[t] = ev.tile([P, BPC], dt.float16, tag="evict",
                                   name=f"s{t}")
                    nc.scalar.activation(
                        s[t][:], pt[:], AF.Identity,
                        bias=bias_sb[t, 3][:, mt:mt + 1],
                        scale=1.0 / G,
                    )
                zt[g] = ev.tile([P, BPC], dt.float16, tag=f"z{g % 2}",
                                name=f"z{g}")
                nc.vector.tensor_mul(zt[g][:], s["o"][:], s["d"][:])
                if g == 1:
                    nc.vector.tensor_add(zt[0][:], zt[0][:], zt[1][:])
                if g == 3:
                    nc.vector.tensor_add(zt[2][:], zt[2][:], zt[3][:])
                    nc.vector.tensor_add(zt[0][:], zt[0][:], zt[2][:])
            nc.tensor.matmul(
                pred_ps[:], masks_sb[:, ts(a, A)], zt[0][:],
                start=(a == 0), stop=(a == A - 1),
            )

        pred_sb = ev.tile([A, BPC], dt.float32, tag="predsb")
        nc.vector.tensor_copy(pred_sb[:], pred_ps[:])
        nc.sync.dma_start(pred_d[:], pred_sb[:])

    nc.compile()
    return nc


def _calibrate(inputs):
    """Host calibration: per-trunk L2-output feature power + max on a
    CAL_ROWS-row subsample. Returns per-trunk (perm, s)."""
    out = {}
    for t, pfx, xk in (("o", "obs", "obs"), ("d", "dlt", "deltas")):
        hc = np.asarray(inputs[xk][:CAL_ROWS], np.float32)
        for l in range(3):
            W = np.asarray(inputs[f"{pfx}_W{l}"], np.float32)
            b = np.asarray(inputs[f"{pfx}_b{l}"], np.float32)
            hc = np.maximum(hc @ W + b, 0.0)
        power = (hc ** 2).mean(0)
        perm = np.argsort(power)
        hmax = hc[:, perm[:Q * P]].max()
        out[t] = (perm, HMARGIN / max(hmax, 1e-9))
    return out


def _prep_inputs(inputs):
    """Host-side layout/dtype prep shared across cores + per-core slices."""
    shared = {}
    cal = _calibrate(inputs)

    for t, pfx in (("o", "obs"), ("d", "dlt")):
        perm, s = cal[t]
        for l in range(3):
            w = np.asarray(inputs[f"{pfx}_W{l}"], np.float32)
            b = np.asarray(inputs[f"{pfx}_b{l}"], np.float32)
            if l == 2:
                w = w[:, perm]
                b = b[perm]
            shared[f"{t}w{l}"] = _tile_weight(w, GR[l])
            bt = _tile_bias(b)
            if l == 2:
                bt[:, :Q] *= s  # fp8 evictions get scale-folded biases
            shared[f"{t}b{l}"] = bt

        w3 = np.asarray(inputs[f"{pfx}_W3"], np.float32)
        b3 = np.asarray(inputs[f"{pfx}_b3"], np.float32)
        if t == "d":
            # permute columns (f,a) -> (a,f) to match obs layout
            w3 = w3.reshape(H, F, A).transpose(0, 2, 1).reshape(H, DOUT)
            b3 = b3.reshape(F, A).T.reshape(DOUT)
        w3 = w3[perm, :]
        # fp16 part: k-blocks Q..15, scaled by G; column order within an
        # action must be (g*NB + j)*128 + m
        w3b = w3[Q * P:, :] * G                         # [NB*128, 16384]
        w3b = w3b.reshape(NB, P, A, GR[3], P)           # [j, k, a, g, m]
        w3b = w3b.transpose(2, 1, 3, 0, 4)              # [a, k, g, j, m]
        w3b = w3b.reshape(A, P, GR[3] * NB * P)
        shared[f"{t}w3b"] = np.ascontiguousarray(w3b.astype(F16))
        # fp8 part: k-blocks 0..Q-1, scaled by G/s; layout [A, P, Q, 4*128]
        w3q = w3[:Q * P, :] * (G / s)                   # [Q*128, 16384]
        assert np.abs(w3q).max() < 240.0, np.abs(w3q).max()
        w3q = w3q.reshape(Q, P, A, GR[3] * P)           # [j, k, a, gm]
        w3q = w3q.transpose(2, 1, 0, 3)                 # [a, k, j, gm]
        w3q = w3q.reshape(A, P, Q * GR[3] * P)
        shared[f"{t}w3q"] = np.ascontiguousarray(w3q.astype(E4M3))
        shared[f"{t}b3"] = _tile_bias(b3)

    masks = np.zeros((P, A, A), np.float32)
    for a in range(A):
        masks[:, a, a] = 1.0
    shared["masks"] = np.ascontiguousarray(masks.reshape(P, A * A).astype(F16))
    shared["scales"] = np.ascontiguousarray(
        np.broadcast_to(np.array([cal["o"][1], cal["d"][1]], np.float32),
                        (P, 2)))

    obsT = np.asarray(inputs["obs"], np.float32).T.astype(F16)    # [256, 4096]
    dltT = np.asarray(inputs["deltas"], np.float32).T.astype(F16)

    in_maps = []
    for c in range(NCORES):
        sl = slice(c * BPC, (c + 1) * BPC)
        m = dict(shared)
        m["xo"] = np.ascontiguousarray(
            obsT[:, sl].reshape(2, P, BPC).transpose(1, 0, 2).reshape(P, 2 * BPC))
        m["xd"] = np.ascontiguousarray(
            dltT[:, sl].reshape(2, P, BPC).transpose(1, 0, 2).reshape(P, 2 * BPC))
        in_maps.append(m)
    return in_maps


_PROGRAM = None


def kernel(**inputs):
    global _PROGRAM, LAST_RESULTS
    from concourse.bass_utils import run_bass_kernel_spmd

    if _PROGRAM is None:
        _PROGRAM = _build_program()
    in_maps = _prep_inputs(inputs)
    res = run_bass_kernel_spmd(_PROGRAM, in_maps, list(range(NCORES)))
    LAST_RESULTS = res
    out = np.empty((B, A), np.float32)
    for c in range(NCORES):
        out[c * BPC:(c + 1) * BPC] = res.results[c]["pred"].T
    return out


# revision 13
# speedup vs baseline: 1.2101x; 1.0075x over previous
"""Trainium2 Bass kernel for nn_BilinearPolicy (dense_mlp).

Math (reference):
  ob = trunk_obs(obs)      : [B,256] -> 2048 -> 2048 -> 2048 -> 16384 (ReLU between)
  dl = trunk_dlt(deltas)   : same shapes, different weights
  pred[b,a] = sum_f ob[b, a*512+f] * dl[b, f*32+a]            : [B, 32]

Strategy:
  * Data-parallel over batch: 8 cores x 512 rows, zero collectives.
  * Feature-major activations on chip ([feat(part), batch(free)]), so the
    torch-layout weights [din, dout] are used directly as matmul lhsT tiles
    and no transposes are ever needed. Inputs are transposed on host.
  * fp16 matmuls with fp32 PSUM accumulation (same PE rate as bf16,
    8x finer mantissa). Biases applied during the
    mandatory PSUM->SBUF eviction on the Scalar engine (Relu / Identity).
  * Mixed-precision L3 (79% of the FLOPs): L2's output features are
    permuted on host (free: permute W2 columns + W3 rows) so the Q coldest
    128-feature blocks (host-calibrated power on a 256-row subsample) come
    first; those blocks are quantized to fp8-e4m3 and contracted with
    DoubleRow matmuls (2 k-blocks per MM at ~0.5 cycles/row). The L3 PSUM
    runs in G*y units (fp16 W3 tiles scaled by G on host; fp8 products
    carry G via the weight scale G/s against the activation scale s);
    evictions apply 1/G before the bias.
  * dl's last-layer weight columns are permuted on host from (f,a) to (a,f)
    ordering, so the bilinear diagonal becomes an elementwise multiply of
    the two trunk outputs + segmented reduction. The z path (evict, mul,
    accumulate) runs in fp16 (2x DVE rate); one one-hot mask matmul per
    action accumulates pred^T [32, 512] in a single PSUM tile.
  * Weight streaming: weights pre-grouped on host so each DMA moves large
    contiguous partition lines, round-robin over the sync + scalar HWDGE
    queues. Small constants go via gpsimd SWDGE.
"""

from contextlib import ExitStack

import numpy as np
import ml_dtypes

B, OBS, H, F, A = 4096, 256, 2048, 512, 32
DOUT = F * A            # 16384
NCORES = 8
BPC = B // NCORES       # 512 batch rows per core
P = 128

KT = [2, 16, 16, 16]    # k-tiles per layer
MT = [16, 16, 16, 128]  # m-tiles per layer
GR = [8, 4, 4, 4]       # m-tiles grouped per weight DMA

Q = 8                   # L3 k-blocks (of 16) quantized to fp8-DoubleRow
NB = 16 - Q             # L3 k-blocks kept in fp16
G = 4096.0              # L3 PSUM scale
CAL_ROWS = 256          # host calibration subsample
HMARGIN = 120.0         # fp8 |h*s| target max (overflow at 240)

F16 = np.float16
E4M3 = ml_dtypes.float8_e4m3   # IEEE-style: max 240, matches TRN FP8_EXP4

# Filled with the BassKernelResults of the most recent run (for test harness).
LAST_RESULTS = None


def _tile_weight(w, G_):
    """[D1, D2] fp32 -> [D2/(128G), 128(k), G*D1] bf16. Slice
    [:, (g*Kt + j)*128 : +128] of group tile mtg is the lhsT for
    k-tile j of m-tile mtg*G+g; every partition line is contiguous."""
    d1, d2 = w.shape
    kt, mt = d1 // P, d2 // P
    wt = w.reshape(kt, P, mt, P).transpose(2, 1, 0, 3)      # [mt, k, j, m]
    wt = wt.reshape(mt // G_, G_, P, kt * P).transpose(0, 2, 1, 3)
    wt = wt.reshape(mt // G_, P, G_ * kt * P)
    return np.ascontiguousarray(wt.astype(F16))


def _tile_bias(b):
    """[D2] fp32 -> [128, D2/128] fp32; column mt holds bias for m-tile mt
    as a per-partition scalar."""
    return np.ascontiguousarray(b.reshape(-1, P).T.astype(np.float32))


def _build_program():
    import concourse.bass as bass
    import concourse.tile as tile
    from concourse import bacc, mybir
    from concourse.bass import ts

    dt = mybir.dt
    AF = mybir.ActivationFunctionType
    DRM = mybir.MatmulPerfMode.DoubleRow

    nc = bacc.Bacc(
        "TRN2",
        target_bir_lowering=False,
        debug=False,
        enable_asserts=True,
        num_devices=NCORES,
    )

    def din(name, shape, dtype):
        return nc.dram_tensor(name, shape, dtype, kind="ExternalInput").ap()

    x_d = {
        "o": din("xo", [P, 2 * BPC], dt.float16),
        "d": din("xd", [P, 2 * BPC], dt.float16),
    }
    w_d = {}
    b_d = {}
    for t in ("o", "d"):
        for l in range(3):
            w_d[t, l] = din(f"{t}w{l}", [MT[l] // GR[l], P, GR[l] * KT[l] * P],
                            dt.float16)
        # L3 split: fp16 part (NB k-blocks) + fp8 DoubleRow part (Q k-blocks)
        w_d[t, "3b"] = din(f"{t}w3b", [A, P, GR[3] * NB * P], dt.float16)
        w_d[t, "3q"] = din(f"{t}w3q", [A, P, Q * GR[3] * P], dt.float8e4)
        for l in range(4):
            b_d[t, l] = din(f"{t}b{l}", [P, MT[l]], dt.float32)
    masks_d = din("masks", [P, A * A], dt.float16)
    scales_d = din("scales", [P, 2], dt.float32)
    pred_d = nc.dram_tensor("pred", [A, BPC], dt.float32, kind="ExternalOutput").ap()

    with tile.TileContext(nc) as tc, ExitStack() as ctx:
        const = ctx.enter_context(tc.tile_pool(name="const", bufs=1))
        wp = ctx.enter_context(tc.tile_pool(name="wp", bufs=4))
        wp3 = ctx.enter_context(tc.tile_pool(name="wp3", bufs=5))
        act = ctx.enter_context(tc.tile_pool(name="act", bufs=1))
        ev = ctx.enter_context(tc.tile_pool(name="ev", bufs=3))
        ps = ctx.enter_context(tc.tile_pool(name="ps", bufs=7, space="PSUM"))
        psp = ctx.enter_context(tc.tile_pool(name="psp", bufs=1, space="PSUM"))

        # weight DMAs round-robin over the sync + scalar HWDGE queues so
        # supply isn't capped by a single queue's descriptor rate
        dma_engs = [nc.sync, nc.scalar]
        rr = [0]

        def wdma(dst, src):
            dma_engs[rr[0] % len(dma_engs)].dma_start(dst, src)
            rr[0] += 1

        # inputs + L0 weights first, split across both HWDGE queues in
        # first-use order so the PE can start within a few us; other small
        # constants go on the gpsimd SWDGE queue
        x_sb = {}
        for t in ("o", "d"):
            x_sb[t] = const.tile([P, 2, BPC], dt.float16,
                                 tag=f"x{t}", name=f"x{t}")
            xsrc = x_d[t].rearrange("p (k n) -> p k n", n=BPC)
            for k in range(2):
                wdma(x_sb[t][:, k, :], xsrc[:, k, :])
        bias_sb = {}
        for t in ("o", "d"):
            for l in range(4):
                bias_sb[t, l] = const.tile([P, MT[l]], dt.float32,
                                           tag=f"b{t}{l}", name=f"b{t}{l}")
                nc.gpsimd.dma_start(bias_sb[t, l][:], b_d[t, l][:])
        masks_sb = const.tile([P, A * A], dt.float16, tag="masks")
        nc.gpsimd.dma_start(masks_sb[:], masks_d[:])
        scales_sb = const.tile([P, 2], dt.float32, tag="scales")
        nc.gpsimd.dma_start(scales_sb[:], scales_d[:])
        scol = {"o": 0, "d": 1}

        # ---- Trunks: layers 0..2 with ReLU, feature-major throughout.
        # The two trunks are interleaved layer-by-layer so the PE has twice
        # the work per phase start, covering the weight-stream warm-up.
        # L2's output features are host-permuted: blocks 0..Q-1 (coldest)
        # are evicted to fp8 with scale s_t, blocks Q..15 to fp16.
        cur = dict(x_sb)
        hq_sb = {}
        for l in range(3):
            for t in ("o", "d"):
                out_t = act.tile([P, MT[l], BPC], dt.float16,
                                 tag=f"h{t}{l % 2}", name=f"h{t}{l}")
                if l == 2:
                    hq_sb[t] = act.tile([P, Q, BPC], dt.float8e4,
                                        tag=f"hq{t}", name=f"hq{t}")
                for mtg in range(MT[l] // GR[l]):
                    wt = wp.tile([P, GR[l] * KT[l] * P], dt.float16,
                                 tag="w0" if l == 0 else "wbig",
                                 name="wt", bufs=4 if l == 0 else 3)
                    wdma(wt[:], w_d[t, l][mtg])
                    for g in range(GR[l]):
                        mt = mtg * GR[l] + g
                        wcol = g
                        pt = ps.tile([P, BPC], dt.float32, tag="mm")
                        for j in range(KT[l]):
                            nc.tensor.matmul(
                                pt[:], wt[:, ts(wcol * KT[l] + j, P)],
                                cur[t][:, j, :],
                                start=(j == 0), stop=(j == KT[l] - 1),
                            )
                        if l == 2 and mt < Q:
                            # Relu(psum*s + s*b) = s*Relu(psum+b) -> fp8
                            nc.scalar.activation(
                                hq_sb[t][:, mt, :], pt[:], AF.Relu,
                                bias=bias_sb[t, 2][:, mt:mt + 1],
                                scale=scales_sb[:, scol[t]:scol[t] + 1],
                            )
                        else:
                            slot = mt if l < 2 else mt - Q
                            nc.scalar.activation(
                                out_t[:, slot, :], pt[:], AF.Relu,
                                bias=bias_sb[t, l][:, mt:mt + 1],
                            )
                cur[t] = out_t
        h = cur

        # interleaved L3 MM sequence: every DoubleRow LDWEIGHTS (no FWL,
        # 256 cols) hides under a long predecessor stream
        seq = []
        for j in range(max(NB, Q // 2)):
            if j < NB:
                seq.append(("b", j))
            if j < Q // 2:
                seq.append(("q", j))

        # ---- Layer 3 + bilinear diagonal, fused per 128-feature tile.
        pred_ps = psp.tile([A, BPC], dt.float32, tag="pred")
        for a in range(A):  # one weight DMA pair per trunk covers the action
            wtb = {}
            wtq = {}
            for t in ("o", "d"):
                wtb[t] = wp3.tile([P, GR[3] * NB * P], dt.float16,
                                  tag="w3b", name=f"w3b{t}")
                wdma(wtb[t][:], w_d[t, "3b"][a])
                wtq[t] = wp3.tile([P, Q, GR[3] * P], dt.float8e4,
                                  tag="w3q", name=f"w3q{t}", bufs=4)
                nc.gpsimd.dma_start(wtq[t][:], w_d[t, "3q"][a].rearrange(
                    "p (q m) -> p q m", q=Q))
            zt = {}
            for g in range(GR[3]):
                mt = a * 4 + g
                s = {}
                for t in ("o", "d"):
                    pt = ps.tile([P, BPC], dt.float32, tag="mm")
                    for i, (kind, j) in enumerate(seq):
                        if kind == "b":
                            nc.tensor.matmul(
                                pt[:], wtb[t][:, ts(g * NB + j, P)],
                                h[t][:, j, :],
                                start=(i == 0), stop=(i == len(seq) - 1),
                            )
                        else:
                            nc.tensor.matmul(
                                pt[:], wtq[t][:, 2 * j:2 * j + 2, ts(g, P)],
                                hq_sb[t][:, 2 * j:2 * j + 2, :],
                                start=(i == 0), stop=(i == len(seq) - 1),
                                perf_mode=DRM,
                            )
                    s[t] = ev.tile([P, BPC], dt.float16, tag="evict",
                                   name=f"s{t}", bufs=2)
                    nc.scalar.activation(
                        s[t][:], pt[:], AF.Identity,
                        bias=bias_sb[t, 3][:, mt:mt + 1],
                        scale=1.0 / G,
                    )
                zt[g] = ev.tile([P, BPC], dt.float16, tag=f"z{g % 2}",
                                name=f"z{g}", bufs=2)
                nc.vector.tensor_mul(zt[g][:], s["o"][:], s["d"][:])
                if g == 1:
                    nc.vector.tensor_add(zt[0][:], zt[0][:], zt[1][:])
                if g == 3:
                    nc.vector.tensor_add(zt[2][:], zt[2][:], zt[3][:])
                    nc.vector.tensor_add(zt[0][:], zt[0][:], zt[2][:])
            nc.tensor.matmul(
                pred_ps[:], masks_sb[:, ts(a, A)], zt[0][:],
                start=(a == 0), stop=(a == A - 1),
            )

        pred_sb = ev.tile([A, BPC], dt.float32, tag="predsb", bufs=1)
        nc.vector.tensor_copy(pred_sb[:], pred_ps[:])
        nc.sync.dma_start(pred_d[:], pred_sb[:])

    nc.compile()
    return nc


def _calibrate(inputs):
    """Host calibration: per-trunk L2-output feature power + max on a
    CAL_ROWS-row subsample. Returns per-trunk (perm, s)."""
    out = {}
    for t, pfx, xk in (("o", "obs", "obs"), ("d", "dlt", "deltas")):
        hc = np.asarray(inputs[xk][:CAL_ROWS], np.float32)
        for l in range(3):
            W = np.asarray(inputs[f"{pfx}_W{l}"], np.float32)
            b = np.asarray(inputs[f"{pfx}_b{l}"], np.float32)
            hc = np.maximum(hc @ W + b, 0.0)
        power = (hc ** 2).mean(0)
        perm = np.argsort(power)
        hmax = hc[:, perm[:Q * P]].max()
        out[t] = (perm, HMARGIN / max(hmax, 1e-9))
    return out


def _prep_inputs(inputs):
    """Host-side layout/dtype prep shared across cores + per-core slices."""
    shared = {}
    cal = _calibrate(inputs)

    for t, pfx in (("o", "obs"), ("d", "dlt")):
        perm, s = cal[t]
        for l in range(3):
            w = np.asarray(inputs[f"{pfx}_W{l}"], np.float32)
            b = np.asarray(inputs[f"{pfx}_b{l}"], np.float32)
            if l == 2:
                w = w[:, perm]
                b = b[perm]
            shared[f"{t}w{l}"] = _tile_weight(w, GR[l])
            bt = _tile_bias(b)
            if l == 2:
                bt[:, :Q] *= s  # fp8 evictions get scale-folded biases
            shared[f"{t}b{l}"] = bt

        w3 = np.asarray(inputs[f"{pfx}_W3"], np.float32)
        b3 = np.asarray(inputs[f"{pfx}_b3"], np.float32)
        if t == "d":
            # permute columns (f,a) -> (a,f) to match obs layout
            w3 = w3.reshape(H, F, A).transpose(0, 2, 1).reshape(H, DOUT)
            b3 = b3.reshape(F, A).T.reshape(DOUT)
        w3 = w3[perm, :]
        # fp16 part: k-blocks Q..15, scaled by G; column order within an
        # action must be (g*NB + j)*128 + m
        w3b = w3[Q * P:, :] * G                         # [NB*128, 16384]
        w3b = w3b.reshape(NB, P, A, GR[3], P)           # [j, k, a, g, m]
        w3b = w3b.transpose(2, 1, 3, 0, 4)              # [a, k, g, j, m]
        w3b = w3b.reshape(A, P, GR[3] * NB * P)
        shared[f"{t}w3b"] = np.ascontiguousarray(w3b.astype(F16))
        # fp8 part: k-blocks 0..Q-1, scaled by G/s; layout [A, P, Q, 4*128]
        w3q = w3[:Q * P, :] * (G / s)                   # [Q*128, 16384]
        assert np.abs(w3q).max() < 240.0, np.abs(w3q).max()
        w3q = w3q.reshape(Q, P, A, GR[3] * P)           # [j, k, a, gm]
        w3q = w3q.transpose(2, 1, 0, 3)                 # [a, k, j, gm]
        w3q = w3q.reshape(A, P, Q * GR[3] * P)
        shared[f"{t}w3q"] = np.ascontiguousarray(w3q.astype(E4M3))
        shared[f"{t}b3"] = _tile_bias(b3)

    masks = np.zeros((P, A, A), np.float32)
    for a in range(A):
        masks[:, a, a] = 1.0
    shared["masks"] = np.ascontiguousarray(masks.reshape(P, A * A).astype(F16))
    shared["scales"] = np.ascontiguousarray(
        np.broadcast_to(np.array([cal["o"][1], cal["d"][1]], np.float32),
                        (P, 2)))

    obsT = np.asarray(inputs["obs"], np.float32).T.astype(F16)    # [256, 4096]
    dltT = np.asarray(inputs["deltas"], np.float32).T.astype(F16)

    in_maps = []
    for c in range(NCORES):
        sl = slice(c * BPC, (c + 1) * BPC)
        m = dict(shared)
        m["xo"] = np.ascontiguousarray(
            obsT[:, sl].reshape(2, P, BPC).transpose(1, 0, 2).reshape(P, 2 * BPC))
        m["xd"] = np.ascontiguousarray(
            dltT[:, sl].reshape(2, P, BPC).transpose(1, 0, 2).reshape(P, 2 * BPC))
        in_maps.append(m)
    return in_maps


_PROGRAM = None


def kernel(**inputs):
    global _PROGRAM, LAST_RESULTS
    from concourse.bass_utils import run_bass_kernel_spmd

    if _PROGRAM is None:
        _PROGRAM = _build_program()
    in_maps = _prep_inputs(inputs)
    res = run_bass_kernel_spmd(_PROGRAM, in_maps, list(range(NCORES)))
    LAST_RESULTS = res
    out = np.empty((B, A), np.float32)
    for c in range(NCORES):
        out[c * BPC:(c + 1) * BPC] = res.results[c]["pred"].T
    return out
